# revision 1
# baseline (speedup 1.0000x reference)
"""Trainium2 kernel for nn_CandidateFinder: LSH/Wu-Manber/Trie-masked top-64
candidate retrieval.

Math: for query (b,i) and key (b,j), the pair is a candidate iff
  sig-match:  sign-pattern of query_up[3,i] equals sign-pattern of key_up[3,j]
  lsh-match:  lsh_hash(query_up[b,i]) == lsh_hash(key_up[b,j])
  inserted:   prefix-6 sign patterns of query_up[0,j] and key_up[0,j] agree
and candidates are ranked by sims = query_up[b,i] . key_up[b,j] descending.

The device kernel fuses all three masks and the similarity into a single
PE matmul per (query,key) block producing
  z = C*(sig_agreement + 2*lsh_onehot_dot + 4*inserted) + sims
with C=1024.  A pair is a candidate iff z >= T (= 70656): matched pairs give
integer mask part 70*C, best non-matched 68*C, and |sims| << C.  Ordering by
z among matched pairs equals ordering by sims.  Per query row the DVE
max/max_index instruction pair extracts the top-8 (value-descending, ties by
lower index — identical to jax.lax.top_k's stable order).  Rows with more
than 8 candidates (8th value >= T) are detected and recomputed on host; for
iid-random inputs the expected candidate count per row is ~0 (an exact
64-bit sign-pattern collision is needed), so this path never triggers in
practice.
"""

import os
import sys

for _p in ("/opt/trn_rl_repo", os.path.expanduser("~/.axon_site/_ro/trn_rl_repo")):
    if os.path.isdir(_p) and _p not in sys.path:
        sys.path.insert(0, _p)

import numpy as np

B, S, D, H = 4, 4096, 64, 16
K_MAX = 64
PREFIX_LEN = 6
LSH_BUCKETS = 64
LSH_BANDWIDTH = 4.0
NEG = np.float32(-1e30)

N_CORES = 8
QN = (B * S) // N_CORES  # 2048 query rows per core
KN = S                   # 4096 keys (replicated)

C_SCALE = 1024.0
W_LSH = 2.0
W_INS = 4.0
# matched: 70*C + sims ; best unmatched: 68*C + sims ; |sims| <= ~260
THRESH = 69.0 * C_SCALE

_CACHE = {}


def _build_nc(reps=1):
    import concourse.bacc as bacc
    import concourse.mybir as mybir
    from concourse import masks
    from concourse.tile import TileContext

    dt = mybir.dt
    AF = mybir.ActivationFunctionType
    OP = mybir.AluOpType

    nc = bacc.Bacc("TRN2", target_bir_lowering=False, debug=False,
                   num_devices=N_CORES)

    qb = nc.dram_tensor("qb", [QN, D], dt.float32, kind="ExternalInput")
    q3 = nc.dram_tensor("q3", [QN, D], dt.float32, kind="ExternalInput")
    kb = nc.dram_tensor("kb", [KN, D], dt.float32, kind="ExternalInput")
    k3 = nc.dram_tensor("k3", [KN, D], dt.float32, kind="ExternalInput")
    wmq = nc.dram_tensor("wmq", [KN, PREFIX_LEN], dt.float32, kind="ExternalInput")
    wmk = nc.dram_tensor("wmk", [KN, PREFIX_LEN], dt.float32, kind="ExternalInput")
    lshw = nc.dram_tensor("lshw", [D, H], dt.float32, kind="ExternalInput")

    v8_out = nc.dram_tensor("v8", [QN, 16], dt.float32, kind="ExternalOutput")
    i8_out = nc.dram_tensor("i8", [QN, 16], dt.uint32, kind="ExternalOutput")

    MAGIC = 12582912.0  # 1.5 * 2**23 : float32 round-to-nearest-int magic
    QT = QN // 128      # 16 query tiles
    KC = KN // 128      # 32 key chunks
    QC = QN // 128      # 16 query chunks

    with TileContext(nc) as tc:
        with (
            tc.tile_pool(name="const", bufs=1) as cst,
            tc.tile_pool(name="feat", bufs=1) as feat,
            tc.tile_pool(name="hsb", bufs=6) as hsb,
            tc.tile_pool(name="eqp", bufs=2) as eqp,
            tc.tile_pool(name="sgtmp", bufs=2) as sgtmp,
            tc.tile_pool(name="prep", bufs=2, space="PSUM") as prep,
        ):
            ident = cst.tile([128, 128], dt.float32)
            masks.make_identity(nc, ident[:])
            w_sb = cst.tile([D, H], dt.float32)
            nc.sync.dma_start(w_sb[:], lshw[:])
            w_bf = cst.tile([D, H], dt.bfloat16)
            nc.scalar.activation(w_bf[:], w_sb[:], AF.Copy)
            ones_16x64 = cst.tile([H, 64], dt.float32)
            nc.vector.memset(ones_16x64[:], 1.0)
            ones6 = cst.tile([PREFIX_LEN, 1], dt.float32)
            nc.vector.memset(ones6[:], 1.0)
            iota_i = cst.tile([64, 1], dt.int32)
            nc.gpsimd.iota(iota_i[:], pattern=[[1, 1]], base=0, channel_multiplier=1)
            iota_f = cst.tile([64, 1], dt.float32)
            nc.scalar.activation(iota_f[:], iota_i[:], AF.Copy)

            # staged inputs: [128, nchunk*64]; chunk j col-block = tokens j*128..j*128+127
            kb_st = feat.tile([128, KC * D], dt.float32)
            k3_st = feat.tile([128, KC * D], dt.float32)
            qb_st = feat.tile([128, QC * D], dt.float32)
            q3_st = feat.tile([128, QC * D], dt.float32)
            wmq_st = feat.tile([128, KC * PREFIX_LEN], dt.float32)
            wmk_st = feat.tile([128, KC * PREFIX_LEN], dt.float32)

            def stage_half(dst, src, d, h, nh):
                ntok = (KC // nh) * 128 if dst in (kb_st, k3_st) else 0
                c0 = h * (ntok // 128) * d
                nc.sync.dma_start(
                    dst[:, c0:c0 + (ntok // 128) * d]
                    .rearrange("p (n d) -> p n d", d=d),
                    src[h * ntok:(h + 1) * ntok].rearrange("(n p) d -> p n d", p=128))

            def stage(dst, src, d):
                nc.sync.dma_start(dst[:].rearrange("p (n d) -> p n d", d=d),
                                  src[:].rearrange("(n p) d -> p n d", p=128))

            # persistent feature tensors
            fk1 = feat.tile([128, KN], dt.bfloat16)   # [0:64] sig(k3) ±1 | [64:128] onehot(kh)
            fk2 = feat.tile([65, KN], dt.bfloat16)    # [0:64] raw kb | [64] 4096*ins
            wq1 = feat.tile([128, QN], dt.bfloat16)   # [0:64] C*sig(q3) | [64:128] 2048*onehot(qh)
            wq2 = feat.tile([65, QN], dt.bfloat16)    # [0:64] raw qb | [64] 1.0
            kbt = feat.tile([D, KN], dt.float32)      # kb^T fp32 (lsh matmul rhs)
            qbt = feat.tile([D, QN], dt.float32)      # qb^T fp32
            sg_q0 = feat.tile([PREFIX_LEN, KN], dt.float32)
            sg_k0 = feat.tile([PREFIX_LEN, KN], dt.float32)
            v8_acc = feat.tile([128, QT * 16], dt.float32)
            i8_acc = feat.tile([128, QT * 16], dt.uint32)

            nc.gpsimd.memset(wq2[64:65, :], 1.0)

            def transpose_group(st, g):
                pt = prep.tile([D, 1024], dt.float32, tag="ps")
                for j in range(8):
                    c = g * 8 + j
                    nc.tensor.transpose(pt[:, j * 128:(j + 1) * 128],
                                        st[:, c * D:(c + 1) * D], ident[:])
                return pt

            def hash_group(xt, onehot_dst, scale2, g, floor_on_dve=False):
                cols = slice(g * 1024, (g + 1) * 1024)
                ph = prep.tile([H, 1024], dt.float32, tag="ps")
                for hh in range(2):
                    c0 = g * 1024 + hh * 512
                    nc.tensor.matmul(ph[:, hh * 512:(hh + 1) * 512], w_sb[:],
                                     xt[:, c0:c0 + 512], start=True, stop=True)
                # floor(proj/4) via round-to-nearest magic
                if floor_on_dve:
                    c1 = hsb.tile([H, 1024], dt.float32, tag="h")
                    nc.vector.tensor_scalar(c1[:], ph[:], 1.0 / LSH_BANDWIDTH, -0.5,
                                            OP.mult, OP.add)
                    c3 = hsb.tile([H, 1024], dt.float32, tag="h")
                    nc.vector.tensor_scalar(c3[:], c1[:], MAGIC, -MAGIC,
                                            OP.add, OP.add)
                else:
                    c1 = hsb.tile([H, 1024], dt.float32, tag="h")
                    nc.scalar.activation(c1[:], ph[:], AF.Copy,
                                         scale=1.0 / LSH_BANDWIDTH, bias=-0.5)
                    c2 = hsb.tile([H, 1024], dt.float32, tag="h")
                    nc.scalar.activation(c2[:], c1[:], AF.Copy, bias=MAGIC)
                    c3 = hsb.tile([H, 1024], dt.float32, tag="h")
                    nc.scalar.activation(c3[:], c2[:], AF.Copy, bias=-MAGIC)
                # fused sum+broadcast: [64, 1024] of per-token code sums
                pb = prep.tile([64, 1024], dt.float32, tag="ps")
                for hh in range(2):
                    nc.tensor.matmul(pb[:, hh * 512:(hh + 1) * 512], ones_16x64[:],
                                     c3[:, hh * 512:(hh + 1) * 512],
                                     start=True, stop=True)
                si = hsb.tile([64, 1024], dt.int32, tag="h")
                nc.scalar.activation(si[:], pb[:], AF.Copy)
                hi = hsb.tile([64, 1024], dt.int32, tag="h")
                nc.vector.tensor_scalar(hi[:], si[:], 63, None, OP.bitwise_and)
                hf = hsb.tile([64, 1024], dt.float32, tag="h")
                nc.scalar.activation(hf[:], hi[:], AF.Copy)
                if scale2 is None:
                    nc.vector.tensor_scalar(onehot_dst[:, cols], hf[:], iota_f[:],
                                            None, OP.is_equal)
                else:
                    nc.vector.tensor_scalar(onehot_dst[:, cols], hf[:], iota_f[:],
                                            scale2, OP.is_equal, OP.mult)

            def key_half_prep(h, floor_on_dve=False):
                stage_half(kb_st, kb, D, h, 2)
                stage_half(k3_st, k3, D, h, 2)
                # wu-manber prefix signs for this half
                wcols = slice(h * (KC // 2) * PREFIX_LEN,
                              (h + 1) * (KC // 2) * PREFIX_LEN)
                nc.sync.dma_start(
                    wmq_st[:, wcols].rearrange("p (n d) -> p n d", d=PREFIX_LEN),
                    wmq[h * (KN // 2):(h + 1) * (KN // 2)]
                    .rearrange("(n p) d -> p n d", p=128))
                nc.sync.dma_start(
                    wmk_st[:, wcols].rearrange("p (n d) -> p n d", d=PREFIX_LEN),
                    wmk[h * (KN // 2):(h + 1) * (KN // 2)]
                    .rearrange("(n p) d -> p n d", p=128))
                for g in (2 * h, 2 * h + 1):
                    pt = transpose_group(kb_st, g)
                    cols = slice(g * 1024, (g + 1) * 1024)
                    nc.scalar.activation(fk2[0:64, cols], pt[:], AF.Copy)
                    nc.scalar.activation(kbt[:, cols], pt[:], AF.Copy)
                for g in (2 * h, 2 * h + 1):
                    pt = transpose_group(k3_st, g)
                    cols = slice(g * 1024, (g + 1) * 1024)
                    nc.scalar.activation(fk1[0:64, cols], pt[:], AF.Sign)
                for g in (2 * h, 2 * h + 1):
                    hash_group(kbt, fk1[64:128, :], None, g, floor_on_dve)
                for g in (2 * h, 2 * h + 1):
                    ptq = prep.tile([PREFIX_LEN, 1024], dt.float32, tag="ps")
                    ptk = prep.tile([PREFIX_LEN, 1024], dt.float32, tag="ps")
                    for j in range(8):
                        c = g * 8 + j
                        nc.tensor.transpose(
                            ptq[:, j * 128:(j + 1) * 128],
                            wmq_st[:, c * PREFIX_LEN:(c + 1) * PREFIX_LEN], ident[:])
                        nc.tensor.transpose(
                            ptk[:, j * 128:(j + 1) * 128],
                            wmk_st[:, c * PREFIX_LEN:(c + 1) * PREFIX_LEN], ident[:])
                    cols = slice(g * 1024, (g + 1) * 1024)
                    nc.scalar.activation(sg_q0[:, cols], ptq[:], AF.Sign)
                    nc.scalar.activation(sg_k0[:, cols], ptk[:], AF.Sign)
                hcols = slice(h * (KN // 2), (h + 1) * (KN // 2))
                eq0 = eqp.tile([PREFIX_LEN, KN // 2], dt.float32, tag="eq0")
                nc.vector.tensor_tensor(eq0[:], sg_q0[:, hcols], sg_k0[:, hcols],
                                        OP.is_equal)
                for g in range(4):
                    gc = slice(g * 512, (g + 1) * 512)
                    kc = slice(h * (KN // 2) + g * 512, h * (KN // 2) + (g + 1) * 512)
                    pc = prep.tile([1, 512], dt.float32, tag="ps")
                    nc.tensor.matmul(pc[:], ones6[:], eq0[:, gc], start=True, stop=True)
                    nc.vector.tensor_scalar(fk2[64:65, kc], pc[:],
                                            float(PREFIX_LEN) - 0.5, W_INS * C_SCALE,
                                            OP.is_ge, OP.mult)

            def query_prep():
                stage(qb_st, qb, D)
                stage(q3_st, q3, D)
                for g in range(QN // 1024):         # qb
                    pt = transpose_group(qb_st, g)
                    cols = slice(g * 1024, (g + 1) * 1024)
                    nc.scalar.activation(wq2[0:64, cols], pt[:], AF.Copy)
                    nc.scalar.activation(qbt[:, cols], pt[:], AF.Copy)
                for g in range(QN // 1024):         # q3
                    pt = transpose_group(q3_st, g)
                    cols = slice(g * 1024, (g + 1) * 1024)
                    sg = sgtmp.tile([64, 1024], dt.float32, tag="sg")
                    nc.scalar.activation(sg[:], pt[:], AF.Sign)
                    nc.scalar.activation(wq1[0:64, cols], sg[:], AF.Copy,
                                         scale=C_SCALE)
                for g in range(QN // 1024):
                    hash_group(qbt, wq1[64:128, :], W_LSH * C_SCALE, g, True)

            with (
                tc.tile_pool(name="zsb", bufs=4) as zsb,
                tc.tile_pool(name="psz", bufs=2, space="PSUM") as psz,
            ):
                def phase_d_half(half, t0=0, t1=QT):
                    for t in range(t0, t1):
                        tcols = slice(t * 128, (t + 1) * 128)
                        z = zsb.tile([128, KN // 2], dt.float32, tag="z")
                        for p in range(2):
                            pz = psz.tile([128, 1024], dt.float32, tag="pz")
                            for n in range(2):
                                kcols = slice(half * 2048 + p * 1024 + n * 512,
                                              half * 2048 + p * 1024 + (n + 1) * 512)
                                nc.tensor.matmul(pz[:, n * 512:(n + 1) * 512],
                                                 wq1[:, tcols], fk1[:, kcols],
                                                 start=True, stop=False)
                            for n in range(2):
                                kcols = slice(half * 2048 + p * 1024 + n * 512,
                                              half * 2048 + p * 1024 + (n + 1) * 512)
                                nc.tensor.matmul(pz[:, n * 512:(n + 1) * 512],
                                                 wq2[:, tcols], fk2[:, kcols],
                                                 start=False, stop=True)
                            nc.scalar.activation(z[:, p * 1024:(p + 1) * 1024],
                                                 pz[:], AF.Copy)
                        ocols = slice(t * 16 + half * 8, t * 16 + half * 8 + 8)
                        nc.vector.max(v8_acc[:, ocols], z[:])
                        nc.vector.max_index(i8_acc[:, ocols], v8_acc[:, ocols], z[:])

                for _rep in range(reps):
                    query_prep()
                    key_half_prep(0, floor_on_dve=True)
                    phase_d_half(0, 0, 4)
                    key_half_prep(1)
                    phase_d_half(0, 4, QT)
                    phase_d_half(1)

            for ob in range(4):
                ts_ = slice(ob * 4 * 128, (ob + 1) * 4 * 128)
                cs_ = slice(ob * 4 * 16, (ob + 1) * 4 * 16)
                nc.sync.dma_start(
                    v8_out[ts_].rearrange("(t p) k -> p t k", p=128),
                    v8_acc[:, cs_].rearrange("p (t k) -> p t k", k=16))
                nc.sync.dma_start(
                    i8_out[ts_].rearrange("(t p) k -> p t k", p=128),
                    i8_acc[:, cs_].rearrange("p (t k) -> p t k", k=16))

    nc.compile()
    return nc


def _get_nc(reps=1):
    key = f"nc{reps}"
    if key not in _CACHE:
        _CACHE[key] = _build_nc(reps)
    return _CACHE[key]


def _reference_numpy(query_up, key_up, lsh_W):
    """Exact-semantics host fallback (only for >8-candidate rows; ~never)."""
    q = np.asarray(query_up, np.float32)
    k = np.asarray(key_up, np.float32)
    W = np.asarray(lsh_W, np.float32)
    qbin = (q > 0)
    kbin = (k > 0)

    def lsh_hash(x):
        proj = x.reshape(-1, D) @ W
        codes = np.floor(proj / LSH_BANDWIDTH).astype(np.int64)
        return (codes.sum(-1) % LSH_BUCKETS).reshape(B, S)

    qh = lsh_hash(q)
    kh = lsh_hash(k)
    inserted = np.all(qbin[0, :, :PREFIX_LEN] == kbin[0, :, :PREFIX_LEN], axis=-1)
    sig_match = np.all(qbin[-1][:, None, :] == kbin[-1][None, :, :], axis=-1)
    trie = sig_match & inserted[None, :]
    out = np.full((B, S, K_MAX), -1, np.int32)
    for b in range(B):
        lsh_m = qh[b][:, None] == kh[b][None, :]
        combined = lsh_m & trie
        sims = q[b] @ k[b].T
        masked = np.where(combined, sims, NEG)
        order = np.argsort(-masked, axis=-1, kind="stable")[:, :K_MAX]
        vals = np.take_along_axis(masked, order, axis=-1)
        out[b] = np.where(vals > NEG, order, -1).astype(np.int32)
    return out


def kernel(query_up, key_up, lsh_W, head_idx=0, **_):
    from concourse.bass_utils import run_bass_kernel_spmd

    q = np.ascontiguousarray(np.asarray(query_up, np.float32))
    k = np.ascontiguousarray(np.asarray(key_up, np.float32))
    W = np.ascontiguousarray(np.asarray(lsh_W, np.float32))

    wmq = np.ascontiguousarray(q[0, :, :PREFIX_LEN])
    wmk = np.ascontiguousarray(k[0, :, :PREFIX_LEN])

    in_maps = []
    for c in range(N_CORES):
        b = c // (N_CORES // B)
        r0 = (c % (N_CORES // B)) * QN
        in_maps.append({
            "qb": np.ascontiguousarray(q[b, r0:r0 + QN]),
            "q3": np.ascontiguousarray(q[B - 1, r0:r0 + QN]),
            "kb": np.ascontiguousarray(k[b]),
            "k3": np.ascontiguousarray(k[B - 1]),
            "wmq": wmq,
            "wmk": wmk,
            "lshw": W,
        })

    nc = _get_nc()
    res = run_bass_kernel_spmd(nc, in_maps, list(range(N_CORES))).results

    out = np.full((B, S, K_MAX), -1, np.int32)
    overflow = False
    for c in range(N_CORES):
        b = c // (N_CORES // B)
        r0 = (c % (N_CORES // B)) * QN
        v16 = res[c]["v8"]
        i16 = res[c]["i8"].astype(np.int32)
        i16 = i16 + (np.arange(16) // 8).astype(np.int32) * (KN // 2)
        order = np.argsort(-v16, axis=1, kind="stable")[:, :8]
        vtop = np.take_along_axis(v16, order, axis=1)
        itop = np.take_along_axis(i16, order, axis=1)
        out[b, r0:r0 + QN, :8] = np.where(vtop >= THRESH, itop, -1)
        if np.any(v16[:, 7] >= THRESH) or np.any(v16[:, 15] >= THRESH):
            overflow = True
    if overflow:
        return _reference_numpy(q, k, W)
    return out



# revision 10
# speedup vs baseline: 11.7733x; 11.7733x over previous
"""Trainium2 kernel for nn_CandidateFinder: LSH/Wu-Manber/Trie-masked top-64
candidate retrieval.

Math: for query (b,i) and key (b,j), the pair is a candidate iff
  inserted:   prefix-6 sign patterns of query_up[0,j] and key_up[0,j] agree
  sig-match:  sign-pattern of query_up[3,i] equals sign-pattern of key_up[3,j]
  lsh-match:  lsh_hash(query_up[b,i]) == lsh_hash(key_up[b,j])
and candidates are ranked by sims = query_up[b,i] . key_up[b,j] descending.

Algorithm (classic filter-then-verify retrieval):
  1. Host evaluates the exact Wu-Manber prefix filter (pure boolean compare)
     and compacts the surviving key set; keys that fail `inserted` can never
     be candidates for ANY query, so only survivors (padded to a multiple of
     128) enter the pairwise phase.  For iid inputs the expected survivor
     count is S/64.
  2. Each of 8 cores takes 2048 query rows (batch-major shard) and, on
     device: computes the exact LSH hash of every query (f32 projection
     matmul identical to the reference, floor via magic rounding, mod-64
     one-hot), the binary-quantize sign features, and the fused mask score
       z[i,j] = sum_d sign(q3_i)_d * sign(k3_j)_d  + 2 * [qh_i == kh_j]
     against the survivor keys via PE matmuls (integers, exact).  A pair is
     a candidate iff z >= 65 (full 64-bit sign match contributes 64; best
     near-miss is 62+2).  A per-query max-reduce over keys flags rows with
     any candidate.
  3. Host: unflagged rows emit -1*64 (no candidate).  Flagged rows (rare;
     requires an exact 64-bit sign collision) are recomputed exactly in
     numpy, including the sims ordering.

Perf notes: queries are staged two-tokens-per-partition ([128, 1024] with
token t and t+1024 stacked), which halves the f32 hash matmul and every
elementwise pass; the LSH floor offset (+1536 = 24*64 == 0 mod 64) keeps
per-dim codes fp16-exact so the bucket-sum matmul runs at 2-byte rate; the
mod-64 one-hot is a single fused DVE tensor_scalar (python_mod, is_equal);
detection is a single PSUM max-reduce per 8 z-tiles.  A short PE warm-up
during input staging lifts the tensor engine out of its low p-state before
the latency-critical matmuls.
"""

import math
import os
import sys

for _p in ("/opt/trn_rl_repo", os.path.expanduser("~/.axon_site/_ro/trn_rl_repo")):
    if os.path.isdir(_p) and _p not in sys.path:
        sys.path.insert(0, _p)

import numpy as np

B, S, D, H = 4, 4096, 64, 16
K_MAX = 64
PREFIX_LEN = 6
LSH_BUCKETS = 64
LSH_BANDWIDTH = 4.0
NEG = np.float32(-1e30)

N_CORES = 8
QN = (B * S) // N_CORES      # 2048 query rows per core
HALF = QN // 2               # 1024: stacked-layout half
MAGIC = 12582912.0           # 1.5 * 2**23 : f32 round-to-nearest-int magic
OFF = 1536.0                 # 24*64: fp16-exact code offset, == 0 (mod 64)
SIGN_SCALE = np.float32(4096.0)  # sign-preserving fp16 pre-scale
THRESH = 64.5                # z >= 65 <=> candidate (z is integer-valued)
N_WARMUP = 4                 # PE p-state warm-up matmuls

_CACHE = {}


def _plan(kt):
    KP = kt * 128
    KPC = min(KP, 512)            # key columns per matmul / psum region
    NKC = KP // KPC               # key chunks
    GT = max(1, 1024 // KP)       # query tiles per psum z-group (<=2 banks)
    NV = 16 * NKC                 # v16 columns
    return KP, KPC, NKC, GT, NV


def _build_nc(kt, n_real):
    """Device program for KP = kt*128 candidate keys, of which the first
    n_real are real (rest padding, masked to z=0)."""
    import concourse.bacc as bacc
    import concourse.mybir as mybir
    from concourse.tile import TileContext

    dt = mybir.dt
    AF = mybir.ActivationFunctionType
    OP = mybir.AluOpType

    KP, KPC, NKC, GT, NV = _plan(kt)

    nc = bacc.Bacc("TRN2", target_bir_lowering=False, debug=False,
                   num_devices=N_CORES)

    # qbw: [128, 1056] f32 = stacked query values (cols 0:1024; partition
    # rows 0:64 hold dims of token t, rows 64:128 dims of token t+1024)
    # | blockdiag(W, W) in cols 1024:1056.
    qbw = nc.dram_tensor("qbw", [128, HALF + 2 * H], dt.float32,
                         kind="ExternalInput")
    # q3s: stacked batch-3 query values, fp16 (sign-preserving scaled),
    # with blockdiag(ones[16,64]) appended in cols HALF:HALF+128 (rows 0:32).
    q3s = nc.dram_tensor("q3s", [128, HALF + 128], dt.float16,
                         kind="ExternalInput")
    # kst: survivor keys of this core's batch, transposed (hash input).
    kst = nc.dram_tensor("kst", [128, KP], dt.float32, kind="ExternalInput")
    # k3t: survivor keys of batch 3, transposed, fp16 scaled (sign input).
    k3t = nc.dram_tensor("k3t", [128, KP], dt.float16, kind="ExternalInput")
    v16 = nc.dram_tensor("v16", [128, NV], dt.float32, kind="ExternalOutput")

    with TileContext(nc) as tc:
        with (
            tc.tile_pool(name="cst", bufs=1) as cst,
            tc.tile_pool(name="feat", bufs=1) as feat,
            tc.tile_pool(name="tmp", bufs=2) as tmp,
            tc.tile_pool(name="psA", bufs=2, space="PSUM") as psA,
            tc.tile_pool(name="psB", bufs=2, space="PSUM") as psB,
            tc.tile_pool(name="psZ", bufs=2, space="PSUM") as psZ,
        ):
            # ---- constants ----
            warm_a = cst.tile([64, 128], dt.float16)
            warm_b = cst.tile([64, 512], dt.float16)
            nc.gpsimd.memset(warm_a[:], 0.0)
            nc.gpsimd.memset(warm_b[:], 0.0)

            iota_i = cst.tile([128, 1], dt.int32)
            nc.gpsimd.iota(iota_i[:], pattern=[[1, 1]], base=0,
                           channel_multiplier=1)
            iota_m = cst.tile([128, 1], dt.int32)
            nc.vector.tensor_scalar(iota_m[:], iota_i[:], 63, None,
                                    OP.bitwise_and)
            iota_f = cst.tile([128, 1], dt.float32)
            nc.scalar.activation(iota_f[:], iota_m[:], AF.Copy)

            # ---- staged inputs ----
            qbw_sb = feat.tile([128, HALF + 2 * H], dt.float32)
            q3s_sb = feat.tile([128, HALF + 128], dt.float16)
            kst_sb = feat.tile([128, KP], dt.float32)
            k3t_sb = feat.tile([128, KP], dt.float16)
            nc.sync.dma_start(qbw_sb[:], qbw[:])
            nc.sync.dma_start(q3s_sb[:], q3s[:])
            nc.sync.dma_start(kst_sb[:], kst[:])
            nc.sync.dma_start(k3t_sb[:], k3t[:])

            # ---- PE p-state warm-up (overlaps staging DMAs) ----
            for _ in range(N_WARMUP):
                pwu = psZ.tile([128, GT * KP], dt.float32, tag="z")
                nc.tensor.matmul(pwu[:, 0:512], warm_a[:], warm_b[:],
                                 start=True, stop=True)

            # ---- query features ----
            wq_sig = feat.tile([128, HALF], dt.float16)   # sign(q3) {-1,+1}
            wq_oh = feat.tile([128, HALF], dt.float16)    # [qh == bucket(p%64)]
            v16_sb = feat.tile([128, NV], dt.float32)

            QCH = 512                                     # stacked cols/chunk
            for h in range(HALF // QCH):
                cols = slice(h * QCH, (h + 1) * QCH)
                # exact f32 LSH projection (identical numerics to reference)
                ph = psA.tile([32, QCH], dt.float32, tag="ph")
                nc.tensor.matmul(ph[:], qbw_sb[:, HALF:HALF + 2 * H],
                                 qbw_sb[:, cols], start=True, stop=True)
                # floor(proj/4) = round(proj/4 - 0.5) via magic; +OFF keeps
                # the per-dim code fp16-exact and == 0 (mod 64) after x16
                c1 = tmp.tile([32, QCH], dt.float32, tag="c1")
                nc.scalar.activation(c1[:], ph[:], AF.Copy,
                                     scale=1.0 / LSH_BANDWIDTH, bias=-0.5)
                c3 = tmp.tile([32, QCH], dt.float16, tag="c3")
                nc.vector.tensor_scalar(c3[:], c1[:], MAGIC, OFF - MAGIC,
                                        OP.add, OP.add)
                # bucket sums, broadcast to 64 rows per stacked half
                pb = psB.tile([128, QCH], dt.float32, tag="pb")
                nc.tensor.matmul(pb[:], q3s_sb[0:32, HALF:HALF + 128],
                                 c3[:], start=True, stop=True)
                # one-hot of (sum mod 64): int chain (hw-proven ops)
                si = tmp.tile([128, QCH], dt.int32, tag="si")
                nc.scalar.activation(si[:], pb[:], AF.Copy)
                hi = tmp.tile([128, QCH], dt.int32, tag="hi")
                nc.vector.tensor_scalar(hi[:], si[:], 63, None,
                                        OP.bitwise_and)
                hf = tmp.tile([128, QCH], dt.float32, tag="hf")
                nc.scalar.activation(hf[:], hi[:], AF.Copy)
                nc.vector.tensor_scalar(wq_oh[:, cols], hf[:], iota_f[:],
                                        None, OP.is_equal)
                # binary-quantize to {-1,+1} signs
                nc.scalar.activation(wq_sig[:, cols], q3s_sb[:, cols],
                                     AF.Sign)

            # ---- key features ----
            kfs = feat.tile([128, KP], dt.float16)   # sign(k3), both halves
            kfo = feat.tile([128, KP], dt.float16)   # 2*[kh == bucket], x2
            nc.scalar.activation(kfs[:], k3t_sb[:], AF.Sign)
            for kc in range(NKC):
                kcols = slice(kc * KPC, (kc + 1) * KPC)
                phk = psA.tile([32, QCH], dt.float32, tag="ph")
                nc.tensor.matmul(phk[:, 0:KPC], qbw_sb[:, HALF:HALF + 2 * H],
                                 kst_sb[:, kcols], start=True, stop=True)
                c1k = tmp.tile([32, KPC], dt.float32, tag="c1k")
                nc.scalar.activation(c1k[:], phk[:, 0:KPC], AF.Copy,
                                     scale=1.0 / LSH_BANDWIDTH, bias=-0.5)
                c3k = tmp.tile([32, KPC], dt.float16, tag="c3k")
                nc.vector.tensor_scalar(c3k[:], c1k[:], MAGIC, OFF - MAGIC,
                                        OP.add, OP.add)
                pbk = psB.tile([128, QCH], dt.float32, tag="pb")
                nc.tensor.matmul(pbk[:, 0:KPC],
                                 q3s_sb[0:32, HALF:HALF + 128], c3k[:],
                                 start=True, stop=True)
                sik = tmp.tile([128, KPC], dt.int32, tag="sik")
                nc.scalar.activation(sik[:], pbk[:, 0:KPC], AF.Copy)
                hik = tmp.tile([128, KPC], dt.int32, tag="hik")
                nc.vector.tensor_scalar(hik[:], sik[:], 63, None,
                                        OP.bitwise_and)
                hfk = tmp.tile([128, KPC], dt.float32, tag="hfk")
                nc.scalar.activation(hfk[:], hik[:], AF.Copy)
                ohk = tmp.tile([128, KPC], dt.float16, tag="ohk")
                nc.vector.tensor_scalar(ohk[:], hfk[:], iota_f[:],
                                        None, OP.is_equal)
                nc.vector.tensor_scalar(kfo[:, kcols], ohk[:], 2.0, None,
                                        OP.mult)
            if n_real < KP:
                # padding keys can never match anything: z = 0 < threshold
                nc.vector.memset(kfs[:, n_real:KP], 0.0)
                nc.vector.memset(kfo[:, n_real:KP], 0.0)

            # ---- pairwise mask scores + per-query candidate detection ----
            # v16 column c = (g*GT + r)*NKC + kc  ->  query tile t = c // NKC
            for g in range(16 // GT):
                zg = psZ.tile([128, GT * KP], dt.float32, tag="z")
                for r in range(GT):
                    t = g * GT + r
                    half_rows = slice(0, 64) if t < 8 else slice(64, 128)
                    tcols = slice((t % 8) * 128, (t % 8) * 128 + 128)
                    for kc in range(NKC):
                        kcols = slice(kc * KPC, (kc + 1) * KPC)
                        zcols = slice((r * NKC + kc) * KPC,
                                      (r * NKC + kc + 1) * KPC)
                        nc.tensor.matmul(zg[:, zcols],
                                         wq_sig[half_rows, tcols],
                                         kfs[half_rows, kcols],
                                         start=True, stop=False)
                        nc.tensor.matmul(zg[:, zcols],
                                         wq_oh[half_rows, tcols],
                                         kfo[half_rows, kcols],
                                         start=False, stop=True)
                nc.vector.tensor_reduce(
                    v16_sb[:, g * GT * NKC:(g + 1) * GT * NKC],
                    zg[:].rearrange("p (t k) -> p t k", k=KPC),
                    mybir.AxisListType.X, OP.max)

            nc.sync.dma_start(v16[:], v16_sb[:])

    nc.compile()
    return nc


def _get_nc(kt=1, n_real=0):
    key = (kt, n_real)
    if key not in _CACHE:
        _CACHE[key] = _build_nc(kt, n_real)
        _CACHE["last"] = _CACHE[key]
    return _CACHE[key]


# query tile t -> token base offset within the core's 2048-row shard:
# tiles 0..7 are stacked rows 0:64 = tokens t*128, tiles 8..15 are rows
# 64:128 = tokens 1024+(t-8)*128.
def _tile_tok_base(t):
    return t * 128 if t < 8 else HALF + (t - 8) * 128


def _exact_rows(q, k, W, sel, rows_by_batch):
    """Exact numpy recompute of flagged query rows (reference semantics),
    restricted to the survivor key set `sel`."""
    out = {}

    def lsh_hash(x):
        proj = x.astype(np.float32) @ W
        codes = np.floor(proj / LSH_BANDWIDTH).astype(np.int64)
        return codes.sum(-1) % LSH_BUCKETS

    kb3 = k[3] > 0
    for b, rows in rows_by_batch.items():
        if not len(rows):
            continue
        kh = lsh_hash(k[b][sel])
        qh = lsh_hash(q[b][rows])
        qb3 = q[3][rows] > 0
        sig = (qb3[:, None, :] == kb3[None, sel, :]).all(-1)
        lsh_m = qh[:, None] == kh[None, :]
        combined = sig & lsh_m
        sims = q[b][rows].astype(np.float32) @ k[b][sel].T
        masked = np.where(combined, sims, NEG)
        order = np.argsort(-masked, axis=-1, kind="stable")
        res = np.full((len(rows), K_MAX), -1, np.int32)
        ncol = min(K_MAX, len(sel))
        vals = np.take_along_axis(masked, order[:, :ncol], axis=-1)
        idx = sel[order[:, :ncol]].astype(np.int32)
        res[:, :ncol] = np.where(vals > NEG, idx, -1)
        out[b] = res
    return out


def kernel(query_up, key_up, lsh_W, head_idx=0, **_):
    from concourse.bass_utils import run_bass_kernel_spmd

    q = np.asarray(query_up, np.float32)
    k = np.asarray(key_up, np.float32)
    W = np.ascontiguousarray(np.asarray(lsh_W, np.float32))

    # Wu-Manber prefix filter (exact boolean compare): key j survives iff
    # prefix-6 signs of query_up[0,j] and key_up[0,j] agree.
    ins = np.all((q[0, :, :PREFIX_LEN] > 0) == (k[0, :, :PREFIX_LEN] > 0),
                 axis=-1)
    sel = np.flatnonzero(ins)
    n_real = len(sel)
    kt = max(1, math.ceil(n_real / 128))
    KP, KPC, NKC, GT, NV = _plan(kt)

    # blockdiag(W, W) for the stacked hash matmul
    w2 = np.zeros((128, 2 * H), np.float32)
    w2[0:D, 0:H] = W
    w2[D:128, H:2 * H] = W

    k3t = np.zeros((128, KP), np.float16)
    if n_real:
        k3s = (k[3][sel].T * SIGN_SCALE).astype(np.float16)
        k3t[0:D, :n_real] = k3s
        k3t[D:128, :n_real] = k3s

    in_maps = []
    _err = np.seterr(over='ignore')
    for c in range(N_CORES):
        b = c // (N_CORES // B)
        r0 = (c % (N_CORES // B)) * QN
        qs = q[b, r0:r0 + QN]                    # [2048, 64]
        qbw = np.empty((128, HALF + 2 * H), np.float32)
        qbw[0:D, 0:HALF] = qs[0:HALF].T
        qbw[D:128, 0:HALF] = qs[HALF:QN].T
        qbw[:, HALF:] = w2
        q3 = q[3, r0:r0 + QN]
        q3s = np.zeros((128, HALF + 128), np.float16)
        q3s[0:D, 0:HALF] = (q3[0:HALF].T * SIGN_SCALE).astype(np.float16)
        q3s[D:128, 0:HALF] = (q3[HALF:QN].T * SIGN_SCALE).astype(np.float16)
        q3s[0:16, HALF:HALF + 64] = 1.0
        q3s[16:32, HALF + 64:HALF + 128] = 1.0
        kst = np.zeros((128, KP), np.float32)
        if n_real:
            kst[0:D, :n_real] = k[b][sel].T
            kst[D:128, :n_real] = k[b][sel].T
        in_maps.append({
            "qbw": np.ascontiguousarray(qbw),
            "q3s": np.ascontiguousarray(q3s),
            "kst": np.ascontiguousarray(kst),
            "k3t": np.ascontiguousarray(k3t),
        })

    np.seterr(**_err)
    nc = _get_nc(kt, n_real)
    res = run_bass_kernel_spmd(nc, in_maps, list(range(N_CORES))).results

    out = np.full((B, S, K_MAX), -1, np.int32)
    rows_by_batch = {}
    for c in range(N_CORES):
        b = c // (N_CORES // B)
        r0 = (c % (N_CORES // B)) * QN
        v16 = res[c]["v16"]                      # [128, NV]
        flag_p, flag_c = np.nonzero(v16 >= THRESH)
        if len(flag_p):
            toks = [r0 + _tile_tok_base(int(cc) // NKC) + int(p)
                    for p, cc in zip(flag_p, flag_c)]
            rows_by_batch.setdefault(b, []).extend(toks)
    if rows_by_batch:
        rows_by_batch = {b: np.array(sorted(set(r)), np.int64)
                         for b, r in rows_by_batch.items()}
        exact = _exact_rows(q, k, W, sel, rows_by_batch)
        for b, rows in rows_by_batch.items():
            out[b, rows] = exact[b]
    return out
